# revision 16
# baseline (speedup 1.0000x reference)
"""Bayesian linear layer (Monte-Carlo reparameterized GEMM) on 8 Trainium2 cores.

y[s,b,o] = sum_i x[b,i] * (w_mu[o,i] + exp(w_lsigma[o,i]) * r1[s,o,i]) + b_mu[o]
           + exp(b_lsigma[o]) * r2[s,o]

Precision split:
    y[s] = (x @ w_mu^T)  +  x @ (E o r1[s])^T  +  bias[s]
           '--- mu term ---'  '--- noise term ---'
The mu term is sample-independent and needs >=fp16 precision; the noise term
is ~10x smaller in magnitude, so fp8(e4m3) suffices -> DoubleRow
(double-pumped, K=256/instruction) fp8 matmuls at 2x the fp16 PE rate.
E = exp(w_lsigma) is folded into r1 on the host and r1 is host-pre-transposed
to [i, o]: the tensor engine runs pure GEMM.

Sharding: 4-way batch x 2-way samples (core c: batch block c%4 of 1024 rows,
sample group c//4 of 32 samples). Unlike pure sample sharding, each core's
mu-GEMM covers only its own 1024 batch rows (28 us instead of 8x-redundant
111 us of PE time), with zero cross-core communication.

Per-core device kernel:
  phase 1: mu-GEMM, 8 b-tiles x 16 fp16 matmuls -> PSUM -> DVE copy into the
           resident fp16 mu buffer (DVE is idle in phase 1; keeping ACT's
           FIFO clear of phase-1 work avoids a cross-engine convoy at the
           phase transition).
  phase 2: per sample pair: 8 b-tiles x 4 k-pair groups x 4 DoubleRow
           matmuls (2 samples x 2 o-halves share the stationary x tile).
           Eviction: ACT copies psum[128,1024] -> yt fp16, DVE adds the
           pre-combined (mu + bias_s) fp16 tile in 2x mode, y (fp16) DMAs
           alternate the sync HWDGE queue and the gpsimd SWDGE queue. The
           (mu + bias_s) prep also runs on DVE two b-tiles ahead.
  host: reassembles the 8 [32, 1024, 1024] fp16 blocks and upcasts to fp32.
"""

import sys

if "/opt/trn_rl_repo" not in sys.path:
    sys.path.insert(0, "/opt/trn_rl_repo")

from contextlib import ExitStack

import ml_dtypes
import numpy as np

import concourse.bass as bass  # noqa: F401
import concourse.tile as tile
from concourse import bacc, mybir
from concourse.bass_utils import run_bass_kernel_spmd

P = 128
N_IN = 1024
N_OUT = 1024
BATCH = 4096
S = 64
NCORES = 8
BSHARD = 4  # batch blocks
SSHARD = 2  # sample groups
SC = S // SSHARD  # 32 samples per core
BB = BATCH // BSHARD  # 1024 batch rows per core
KT = N_IN // P  # 8 k-tiles (fp16 mu-GEMM)
KK = KT // 2  # 4 k-pairs (fp8 DoubleRow)
BT = BB // P  # 8 b-tiles per core
BT_FULL = BATCH // P  # 32 b-tiles in the full batch
OW = 512
OH = N_OUT // OW  # 2 o-halves

F32 = mybir.dt.float32
F16 = mybir.dt.float16
FP8 = mybir.dt.float8e4
DR = mybir.MatmulPerfMode.DoubleRow

NP_FP8 = ml_dtypes.float8_e4m3

_CACHE = {}


def build_bass():
    nc = bacc.Bacc("TRN2", target_bir_lowering=False, debug=False)

    # host-marshalled layouts (see _prep); b indexes the core's 1024-row block
    #   xbf[bt, p, k, b]  = fp16(x)[bt*128+b, k*128+p]        (mu-GEMM lhsT)
    #   wmuT[p, k, o]     = fp16(w_mu)[o, k*128+p]            (mu-GEMM rhs,
    #                       exact SBUF order -> one linear 16KB/partition DMA)
    #   x8[p, k, b]       = e4m3(x)[b, k*128+p]               (noise lhsT)
    #   r18[s, p, k, o]   = e4m3(E*r1)[s, o, k*128+p]         (noise rhs)
    #   biass[s, p, o]    = fp16 bias, pre-broadcast along p
    xbf = nc.dram_tensor("xbf", [BT, P, KT, P], F16, kind="ExternalInput").ap()
    wmuT = nc.dram_tensor("wmuT", [P, KT, N_OUT], F16, kind="ExternalInput").ap()
    x8 = nc.dram_tensor("x8", [P, KT, BB], FP8, kind="ExternalInput").ap()
    r18 = nc.dram_tensor("r18", [SC, P, KT, N_OUT], FP8, kind="ExternalInput").ap()
    biass = nc.dram_tensor("biass", [SC, P, N_OUT], F16, kind="ExternalInput").ap()
    y = nc.dram_tensor("y", [SC, BB, N_OUT], F16, kind="ExternalOutput").ap()

    with tile.TileContext(nc) as tc, ExitStack() as ctx:
        const = ctx.enter_context(tc.tile_pool(name="const", bufs=1))
        xbf_pool = ctx.enter_context(tc.tile_pool(name="xbf", bufs=3))
        r1_pool = ctx.enter_context(tc.tile_pool(name="r1", bufs=6))
        y_pool = ctx.enter_context(tc.tile_pool(name="yp", bufs=10))
        mb_pool = ctx.enter_context(tc.tile_pool(name="mb", bufs=8))
        bias_pool = ctx.enter_context(tc.tile_pool(name="bias", bufs=6))
        pm_pool = ctx.enter_context(tc.tile_pool(name="pm", bufs=4, space="PSUM"))

        wmu_sb = const.tile([P, KT, N_OUT], F16)  # 16 KB/partition
        x8_sb = const.tile([P, KT, BB], FP8)  # 8 KB/partition
        mu_sb = const.tile([P, BT, N_OUT], F16)  # 16 KB/partition

        def load_slab(s, q=None):
            slab = r1_pool.tile([P, KT, N_OUT], FP8, tag="r1", name=f"r1_{s}")
            (q or nc.gpsimd).dma_start(slab[:], r18[s])
            return slab

        def load_bias(s):
            bm = bias_pool.tile([P, N_OUT], F16, tag="bias", name=f"bias_{s}")
            nc.gpsimd.dma_start(bm[:], biass[s])
            return bm

        # Prologue is chip-HBM-bound (~7.5 MB/core must land before the noise
        # phase; ~6 MB of it before phase 1 ends), so overlap it with PE work:
        #  - wmu is split into per-k DMAs interleaved just-in-time with the
        #    pool-paced xbf stream on sync, so bt0's k-loop starts after only
        #    ~0.5 MB has landed;
        #  - x8 + slab0/1 ride the gpsimd queue early; slab2/3 go on sync
        #    behind the whole xbf stream (needed only by ~64 us).
        nc.sync.dma_start(wmu_sb[:, 0, :], wmuT[:, 0, :])
        bias_t = {0: load_bias(0), 1: load_bias(1)}
        slabs = {}

        # ---- phase 1: mu-GEMM (fp16), mu_sb[:, bt, :] = (x @ w_mu^T)[bt] ----
        for bt in range(BT):
            xt = xbf_pool.tile([P, KT, P], F16, tag="xt")
            nc.sync.dma_start(xt[:], xbf[bt])
            if bt == 0:
                # remaining wmu k-slices: emitted BEFORE bt0's matmuls (reads
                # must follow writes in emission order for Tile dep tracking),
                # but behind xbf0 in sync-queue order so bt0 starts early
                for k in range(1, KT):
                    nc.sync.dma_start(wmu_sb[:, k, :], wmuT[:, k, :])
            elif bt == 1:
                nc.gpsimd.dma_start(x8_sb[:], x8)
            elif bt == 2:
                slabs[0] = load_slab(0)
                slabs[1] = load_slab(1)
            pm = pm_pool.tile([P, N_OUT], F32, tag="pm", name="mu")
            for k in range(KT):
                lhsT = xt[:, k, :]
                for oh in range(OH):
                    nc.tensor.matmul(
                        pm[:, oh * OW : (oh + 1) * OW],
                        lhsT,
                        wmu_sb[:, k, oh * OW : (oh + 1) * OW],
                        start=(k == 0),
                        stop=(k == KT - 1),
                    )
            nc.vector.tensor_copy(mu_sb[:, bt, :], pm[:])
        slabs[2] = load_slab(2, nc.sync)
        slabs[3] = load_slab(3, nc.sync)

        # ---- phase 2: fp8 DoubleRow noise GEMMs, 2 samples interleaved ----
        def prep_mubias(j, s, bt):
            mb = mb_pool.tile([P, N_OUT], F16, tag="mb", name=f"mb_{j}_{bt % 4}")
            nc.vector.tensor_add(mb[:], mu_sb[:, bt, :], bias_t[s][:])
            return mb

        for sp in range(SC // 2):
            s0 = 2 * sp
            mbs = {}
            for bt0 in range(2):  # prologue preps for bt 0,1
                for j in range(2):
                    mbs[(j, bt0)] = prep_mubias(j, s0 + j, bt0)
            for bt in range(BT):
                pms = {}
                for j in range(2):
                    pms[j] = pm_pool.tile([P, N_OUT], F32, tag="pm", name=f"n{j}")
                for kk in range(KK):
                    lhsT = x8_sb[:, 2 * kk : 2 * kk + 2, bt * P : (bt + 1) * P]
                    for j in range(2):
                        rsl = slabs[s0 + j]
                        for oh in range(OH):
                            nc.tensor.matmul(
                                pms[j][:, oh * OW : (oh + 1) * OW],
                                lhsT,
                                rsl[:, 2 * kk : 2 * kk + 2, oh * OW : (oh + 1) * OW],
                                start=(kk == 0),
                                stop=(kk == KK - 1),
                                perf_mode=DR,
                            )
                if bt + 2 < BT:
                    for j in range(2):
                        mbs[(j, bt + 2)] = prep_mubias(j, s0 + j, bt + 2)
                for j in range(2):
                    s = s0 + j
                    yt = y_pool.tile([P, N_OUT], F16, tag="y")
                    nc.scalar.copy(yt[:], pms[j][:])
                    nc.vector.tensor_add(yt[:], yt[:], mbs.pop((j, bt))[:])
                    yq = nc.sync if (bt + j) % 2 == 0 else nc.gpsimd
                    yq.dma_start(y[s, bt * P : (bt + 1) * P, :], yt[:])
                # prefetch 2 pairs ahead of consumption (slab pool holds 6)
                if bt == 0 and s0 + 4 < SC:
                    slabs[s0 + 4] = load_slab(s0 + 4)
                elif bt == 2 and s0 + 5 < SC:
                    slabs[s0 + 5] = load_slab(s0 + 5)
                elif bt == 4 and s0 + 2 < SC:
                    bias_t[s0 + 2] = load_bias(s0 + 2)
                elif bt == 6 and s0 + 3 < SC:
                    bias_t[s0 + 3] = load_bias(s0 + 3)

    nc.compile()
    return nc


def _get_nc():
    if "nc" not in _CACHE:
        _CACHE["nc"] = build_bass()
    return _CACHE["nc"]


def _prep(x, w_mu, w_lsigma, b_mu, b_lsigma, r1, r2):
    """Host-side marshalling (layout/dtype only; the GEMMs stay on device)."""
    bias1 = (b_mu[None, :] + np.exp(b_lsigma)[None, :] * r2).astype(np.float16)
    bias = np.ascontiguousarray(np.broadcast_to(bias1[:, None, :], (S, P, N_OUT)))

    xT = np.ascontiguousarray(x.T)  # [i, b]
    xbf = xT.astype(np.float16).reshape(KT, P, BT_FULL, P).transpose(2, 1, 0, 3).copy()
    x8 = xT.astype(NP_FP8).reshape(KT, P, BATCH).transpose(1, 0, 2).copy()
    wmuT = (
        np.ascontiguousarray(w_mu.T)
        .astype(np.float16)
        .reshape(KT, P, N_OUT)
        .transpose(1, 0, 2)  # [p, k, o] = SBUF layout, linear load
        .copy()
    )
    # noise rhs: fold E into r1, cast fp8, transpose [s, o, i] -> [s, p, k, o]
    noisew = (np.exp(w_lsigma)[None, :, :] * r1).astype(np.float32)
    r18_soi = noisew.astype(NP_FP8)  # [s, o, i]
    r18 = (
        r18_soi.view(np.uint8)
        .transpose(0, 2, 1)  # [s, i, o]
        .reshape(S, KT, P, N_OUT)
        .transpose(0, 2, 1, 3)  # [s, p, k, o]
        .copy()
        .view(NP_FP8)
    )
    return xbf, wmuT, x8, r18, bias


def make_in_maps(xbf, wmuT, x8, r18, bias):
    in_maps = []
    for c in range(NCORES):
        bb = c % BSHARD
        sg = c // BSHARD
        ssl = slice(sg * SC, (sg + 1) * SC)
        in_maps.append(
            {
                "xbf": np.ascontiguousarray(xbf[bb * BT : (bb + 1) * BT]),
                "wmuT": wmuT,
                "x8": np.ascontiguousarray(x8[:, :, bb * BB : (bb + 1) * BB]),
                "r18": np.ascontiguousarray(r18[ssl]),
                "biass": np.ascontiguousarray(bias[ssl]),
            }
        )
    return in_maps


def assemble(results):
    """Stitch the 8 per-core [SC, BB, N_OUT] fp16 blocks into the full fp32 y."""
    out = np.empty((S, BATCH, N_OUT), dtype=np.float32)
    for c in range(NCORES):
        bb = c % BSHARD
        sg = c // BSHARD
        out[sg * SC : (sg + 1) * SC, bb * BB : (bb + 1) * BB, :] = results[c]["y"]
    return out


def kernel(x, w_mu, w_lsigma, b_mu, b_lsigma, r1, r2, N_samples):
    x = np.asarray(x, dtype=np.float32)
    w_mu = np.asarray(w_mu, dtype=np.float32)
    w_lsigma = np.asarray(w_lsigma, dtype=np.float32)
    b_mu = np.asarray(b_mu, dtype=np.float32)
    b_lsigma = np.asarray(b_lsigma, dtype=np.float32)
    r1 = np.asarray(r1, dtype=np.float32)
    r2 = np.asarray(r2, dtype=np.float32)
    assert x.shape == (BATCH, N_IN) and r1.shape == (S, N_OUT, N_IN)

    xbf, wmuT, x8, r18, bias = _prep(x, w_mu, w_lsigma, b_mu, b_lsigma, r1, r2)
    nc = _get_nc()
    in_maps = make_in_maps(xbf, wmuT, x8, r18, bias)
    res = run_bass_kernel_spmd(nc, in_maps, core_ids=list(range(NCORES)))
    return assemble(res.results)
